# revision 25
# baseline (speedup 1.0000x reference)
"""Cross-attention kernel for Trainium2, SPMD across 8 NeuronCores.

Problem: B=4, N=M=2048, QD=1024, CD=768, H=8, DH=64, INNER=512 (f32).
  q = x @ Wq; k = ctx @ Wk; v = ctx @ Wv
  out = softmax(q k^T / sqrt(DH)) v @ Wo + bo

Sharding: batch x query-halves -> 8 shards. Core c handles batch c//2,
query rows (c%2)*1024:(c%2+1)*1024, with that batch's full context.
Weights replicated, cast to bf16 on host (scale folded into Wq).

Per-core dataflow (host pre-arranges every input into its exact SBUF
layout so each DMA is a flat contiguous copy; input DMAs split across
the SP and ACT hardware-DGE queues for parallel issue):
  qT = (scale*Wq)^T @ xT       [INNER, n]   bf16
  kT = Wk^T @ ctxT             [INNER, m]   bf16
  v  = ctxT^T @ Wv             [m, INNER]   bf16
  sT_h = kT_h^T q_h            [m, n] scores; head pairs share the PE
     via 64-row tiling.
  E = exp(sT) on ScalarE, [128,1024] per head per double-slot.
  Softmax denominators WITHOUT PE ones-matmuls: DVE folds the 8 E
     tiles of a group elementwise (bf16), GpSimd partition_all_reduce
     sums across partitions (f32), DVE reciprocal -> rb. This frees a
     full third of the attention PE streaming vs the ones-matmul way.
  O'_pair = v^T @ E per head pair via two concurrent 64-column tiles.
  O = O' * rb                  packed per INNER tile
  out = sum_j O_j^T @ Wo_j + bo (psum-drain DVE add, bf16 out DMA,
     host converts back to f32).

Schedule: 64 double-slots, groups ordered nb-major (all four head
pairs for query block 0, then block 1) so kT/v are produced once in
the front half and the back half is filled with the first block's
output projections. A.V lags scores by LAG_D=12 slots. Fillers are
placed so the PE slot period tracks ScalarE's exp pace (the binding
engine), and DVE fold work is spread one op per slot.
"""

import numpy as np

B, N, M = 4, 2048, 2048
QD, CD = 1024, 768
H, DH = 8, 64
INNER = H * DH  # 512
NS = 1024  # query rows per core
SCALE = DH ** -0.5

_CACHED_NC = None


def build_nc():
    import concourse.bacc as bacc
    import concourse.mybir as mybir
    import concourse.tile as tile
    from concourse import bass_isa

    f32 = mybir.dt.float32
    bf16 = mybir.dt.bfloat16
    FT = mybir.ActivationFunctionType
    AluOp = mybir.AluOpType
    ROp = bass_isa.ReduceOp

    nc = bacc.Bacc(None)
    # flat, host-prearranged inputs: [128 partitions, k-chunks * cols]
    xT0_d = nc.dram_tensor("xT0", (128, 8 * 512), bf16, kind="ExternalInput")
    xT1_d = nc.dram_tensor("xT1", (128, 8 * 512), bf16, kind="ExternalInput")
    ctx_d = [nc.dram_tensor(f"ctx{i}", (128, 6 * 512), bf16,
                            kind="ExternalInput") for i in range(4)]
    Wq_d = nc.dram_tensor("Wq", (128, 8 * 512), bf16, kind="ExternalInput")
    Wk_d = nc.dram_tensor("Wk", (128, 6 * 512), bf16, kind="ExternalInput")
    Wv_d = nc.dram_tensor("Wv", (128, 6 * 512), bf16, kind="ExternalInput")
    Wo_d = nc.dram_tensor("Wo", (128, 4 * 1024), bf16, kind="ExternalInput")
    bo_d = nc.dram_tensor("bo", (128, QD), bf16, kind="ExternalInput")
    out_d = nc.dram_tensor("out", (NS, QD), bf16, kind="ExternalOutput")

    KQ = QD // 128     # 8 k-tiles, q projection
    KC = CD // 128     # 6 k-tiles, k/v projections
    NI = INNER // 128  # 4 partition tiles of INNER (head pairs)
    MT = M // 128      # 16 context m-tiles
    NB = NS // 512     # 2 query blocks
    LAG_D = 14         # A.V lag in double-slots

    with tile.TileContext(nc) as tc, nc.allow_low_precision(
            "bf16 partial sums for softmax denominators; final reduce in f32"):
        with (
            tc.tile_pool(name="w", bufs=1) as wp,
            tc.tile_pool(name="a", bufs=1) as ap,
            tc.tile_pool(name="e", bufs=30) as ep,
            tc.tile_pool(name="ac", bufs=4) as accp,
            tc.tile_pool(name="h", bufs=2) as hp,
            tc.tile_pool(name="r", bufs=2) as rp,
            tc.tile_pool(name="rb", bufs=4) as rbp,
            tc.tile_pool(name="o", bufs=4) as op_,
            tc.tile_pool(name="pp", bufs=2, space="PSUM") as pp,
            tc.tile_pool(name="po", bufs=2, space="PSUM") as ppo,
            tc.tile_pool(name="pss", bufs=2, space="PSUM") as pps,
        ):
            # ---- input DMAs: flat copies; first-needed first.
            # sync (SP) queue carries the query-side stream, scalar (ACT)
            # queue carries the context-weight stream in parallel.
            wq_sb = wp.tile([128, 8 * 512], bf16, tag="wq", name="wq")
            xT_h = [wp.tile([128, 8 * 512], bf16, tag=f"xs{i}", name=f"xs{i}")
                    for i in range(2)]
            wk_sb = wp.tile([128, 6 * 512], bf16, tag="wk", name="wk")
            ctx_q = [wp.tile([128, 6 * 512], bf16, tag=f"cs{i}", name=f"cs{i}")
                     for i in range(4)]
            wv_sb = wp.tile([128, 6 * 512], bf16, tag="wv", name="wv")
            wo_sb = wp.tile([128, 4 * 1024], bf16, tag="wo", name="wo")
            bo_sb = wp.tile([128, QD], bf16, tag="bo", name="bo_sb")

            # input DMAs split across the two hardware-DGE queues (SP and
            # ACT), ordered by first use; the ACT queue finishes its issues
            # well before the first exp is ready to run.
            nc.sync.dma_start(wq_sb[:], Wq_d[:])
            nc.sync.dma_start(xT_h[0][:], xT0_d[:])
            nc.scalar.dma_start(wk_sb[:], Wk_d[:])
            nc.scalar.dma_start(ctx_q[0][:], ctx_d[0][:])
            nc.sync.dma_start(ctx_q[1][:], ctx_d[1][:])
            nc.scalar.dma_start(wv_sb[:], Wv_d[:])
            nc.sync.dma_start(ctx_q[2][:], ctx_d[2][:])
            nc.scalar.dma_start(ctx_q[3][:], ctx_d[3][:])
            nc.sync.dma_start(xT_h[1][:], xT1_d[:])
            nc.scalar.dma_start(wo_sb[:], Wo_d[:])
            nc.scalar.dma_start(bo_sb[:], bo_d[:])

            def wqs(k, j):
                return wq_sb[:, k * INNER + j * 128:k * INNER + (j + 1) * 128]

            def xts(k, nb):
                return xT_h[nb][:, k * 512:(k + 1) * 512]

            def wks(k, j):
                return wk_sb[:, k * INNER + j * 128:k * INNER + (j + 1) * 128]

            def ctxs(k, lo, sz):
                q, l2 = divmod(lo, M // 4)
                return ctx_q[q][:, k * (M // 4) + l2:k * (M // 4) + l2 + sz]

            def wvs(k):
                return wv_sb[:, k * INNER:(k + 1) * INNER]

            def wos(j, qb):
                return wo_sb[:, j * QD + qb * 512:j * QD + (qb + 1) * 512]

            # persistent activations
            qT = [[ap.tile([128, 512], bf16, tag=f"qT{j}_{nb}",
                           name=f"qT{j}_{nb}") for nb in range(NB)]
                  for j in range(NI)]
            kT = [[ap.tile([128, 512], bf16, tag=f"kT{j}_{mb}",
                           name=f"kT{j}_{mb}") for mb in range(4)]
                  for j in range(NI)]
            v = [ap.tile([128, INNER], bf16, tag=f"v{t}", name=f"v{t}")
                 for t in range(MT)]
            On = [ap.tile([128, NS], bf16, tag=f"On{j}", name=f"On{j}")
                  for j in range(NI)]

            def emit_qT(j, nb):
                ps = pp.tile([128, 512], f32, tag="pp", name="pp")
                for k in range(KQ):
                    nc.tensor.matmul(ps[:], wqs(k, j), xts(k, nb),
                                     start=(k == 0), stop=(k == KQ - 1))
                nc.vector.tensor_copy(qT[j][nb][:], ps[:])

            def emit_kT(j, mb):
                ps = pp.tile([128, 512], f32, tag="pp", name="pp")
                for k in range(KC):
                    nc.tensor.matmul(ps[:], wks(k, j), ctxs(k, mb * 512, 512),
                                     start=(k == 0), stop=(k == KC - 1))
                nc.vector.tensor_copy(kT[j][mb][:], ps[:])

            def emit_v(t):
                ps = pp.tile([128, 512], f32, tag="pp", name="pp")
                for k in range(KC):
                    nc.tensor.matmul(ps[:], ctxs(k, t * 128, 128), wvs(k),
                                     start=(k == 0), stop=(k == KC - 1))
                nc.vector.tensor_copy(v[t][:], ps[:])

            # groups nb-major: kT/v land in the front half, the back half
            # is topped up with query-block-0 output projections
            groups = [(j, nb) for nb in range(NB) for j in range(NI)]
            NDS = len(groups) * (MT // 2)  # 64 double-slots
            E = {}     # slot -> (E_h0, E_h1) [128, 1024] bf16
            PO = {}    # group idx -> [128, 512] psum (both heads packed)
            ACC = {}   # group idx -> [acc_h0, acc_h1] bf16 partial sums
            RB = {}    # group idx -> [128, 512] f32 reciprocal denominators

            def emit_scores(s):
                j, nb = groups[s // 8]
                p = s % 8
                psab = [pps.tile([128, 1024], f32, tag="pss", name="pss")
                        for _ in range(2)]
                for dt in range(2):
                    t = 2 * p + dt
                    for hh in range(2):
                        nc.tensor.matmul(
                            psab[hh][:, dt * 512:(dt + 1) * 512],
                            kT[j][t // 4][hh * 64:(hh + 1) * 64,
                                          (t % 4) * 128:(t % 4 + 1) * 128],
                            qT[j][nb][hh * 64:(hh + 1) * 64, :],
                            start=True, stop=True)
                es = []
                for hh in range(2):
                    e = ep.tile([128, 1024], bf16, tag="E", name="E")
                    nc.scalar.activation(e[:], psab[hh][:], FT.Exp)
                    es.append(e)
                E[s] = es

            def emit_av(a):
                gi = a // 8
                j, nb = groups[gi]
                p = a % 8
                if p == 0:
                    PO[gi] = ppo.tile([128, 512], f32, tag="po", name="po")
                po = PO[gi]
                for dt in range(2):
                    t = 2 * p + dt
                    st, sp_ = (t == 0), (t == MT - 1)
                    for hh in range(2):
                        h = 2 * j + hh
                        nc.tensor.matmul(
                            po[hh * 64:(hh + 1) * 64, :],
                            v[t][:, h * 64:(h + 1) * 64],
                            E[a][hh][:, dt * 512:(dt + 1) * 512],
                            start=st, stop=sp_, skip_group_check=True,
                            tile_position=(0, 64 * hh))
                del E[a]

            # denominator fold ops, spread one-ish per slot:
            #  pair:  acc = E[8g] + E[8g+1]          (slot 8g+2)
            #  add i: acc += E[8g+i], i=2..7         (slot 8g+i+1)
            #  half:  hsum = acc[:, :512]+acc[:, 512:]; r = allreduce(hsum)
            #  recip: rb[0:64] = 1/r_h0; rb[64:128] = 1/r_h1
            FOLD = {}
            for g in range(len(groups)):
                FOLD.setdefault(8 * g + 2, []).append(("pair", g))
                for i in range(2, 8):
                    FOLD.setdefault(8 * g + i + 1, []).append(("add", g, i))
                FOLD.setdefault(8 * g + 9, []).append(("half", g))
                FOLD.setdefault(8 * g + 10, []).append(("recip", g))

            def run_fold(step):
                kind, g = step[0], step[1]
                if kind == "pair":
                    acc = [accp.tile([128, 1024], bf16, tag="acc", name="acc")
                           for _ in range(2)]
                    for hh in range(2):
                        nc.vector.tensor_tensor(
                            acc[hh][:], E[8 * g][hh][:], E[8 * g + 1][hh][:],
                            op=AluOp.add)
                    ACC[g] = acc
                elif kind == "add":
                    i = step[2]
                    for hh in range(2):
                        nc.vector.tensor_tensor(
                            ACC[g][hh][:], ACC[g][hh][:], E[8 * g + i][hh][:],
                            op=AluOp.add)
                elif kind == "half":
                    hs = [hp.tile([128, 512], f32, tag="hs", name="hs")
                          for _ in range(2)]
                    rr = [rp.tile([128, 512], f32, tag="r", name="r")
                          for _ in range(2)]
                    for hh in range(2):
                        nc.vector.tensor_tensor(
                            hs[hh][:], ACC[g][hh][:, 0:512],
                            ACC[g][hh][:, 512:1024], op=AluOp.add)
                        nc.gpsimd.partition_all_reduce(
                            rr[hh][:], hs[hh][:], channels=128,
                            reduce_op=ROp.add)
                    ACC[g + 1000] = rr  # stash
                    del ACC[g]
                elif kind == "recip":
                    # full-tile recip: the custom DVE op breaks on
                    # partition-offset slices
                    rr = ACC.pop(g + 1000)
                    rbs = [rbp.tile([128, 512], f32, tag="rb", name="rb")
                           for _ in range(2)]
                    nc.vector.reciprocal_approx_fast(rbs[0][:], rr[0][:])
                    nc.vector.reciprocal_approx_fast(rbs[1][:], rr[1][:])
                    RB[g] = rbs

            def emit_norm(gi):
                j, nb = groups[gi]
                for hh in range(2):
                    sl = slice(hh * 64, (hh + 1) * 64)
                    nc.vector.tensor_tensor(
                        On[j][sl, nb * 512:(nb + 1) * 512], PO[gi][sl, :],
                        RB[gi][hh][sl, :], op=AluOp.mult)
                del PO[gi], RB[gi]

            def emit_final(nt, qb):
                pf = pp.tile([128, 512], f32, tag="pp", name="pf")
                for j in range(NI):
                    nc.tensor.matmul(
                        pf[:], On[j][:, nt * 128:(nt + 1) * 128],
                        wos(j, qb), start=(j == 0), stop=(j == NI - 1))
                ot = op_.tile([128, 512], bf16, tag="ot", name="ot")
                nc.vector.tensor_tensor(
                    ot[:], pf[:], bo_sb[:, qb * 512:(qb + 1) * 512],
                    op=AluOp.add)
                nc.sync.dma_start(
                    out_d[nt * 128:(nt + 1) * 128,
                          qb * 512:(qb + 1) * 512], ot[:])

            # fillers: slot -> emit thunks (deadlines derived from the
            # nb-major group order and LAG_D; see transcript analysis)
            filler = {
                0: [(emit_v, (1,)), (emit_kT, (0, 2))],
                1: [(emit_v, (2,)), (emit_kT, (0, 3))],
                2: [(emit_v, (3,)), (emit_kT, (1, 0))],
                3: [(emit_v, (4,)), (emit_qT, (1, 0))],
                4: [(emit_v, (5,)), (emit_kT, (1, 1))],
                5: [(emit_v, (6,)), (emit_kT, (1, 2))],
                6: [(emit_v, (7,)), (emit_kT, (1, 3))],
                7: [(emit_v, (8,))],
                8: [(emit_v, (9,))],
                9: [(emit_v, (10,)), (emit_kT, (2, 0))],
                10: [(emit_v, (11,)), (emit_qT, (2, 0))],
                11: [(emit_v, (12,)), (emit_kT, (2, 1))],
                12: [(emit_v, (13,)), (emit_kT, (2, 2))],
                13: [(emit_v, (14,)), (emit_kT, (2, 3))],
                14: [(emit_v, (15,))],
                15: [(emit_kT, (3, 0))],
                16: [(emit_kT, (3, 1))],
                17: [(emit_kT, (3, 2))],
                18: [(emit_kT, (3, 3))],
                19: [(emit_qT, (3, 0))],
                27: [(emit_qT, (0, 1))],
                35: [(emit_qT, (1, 1))],
                43: [(emit_qT, (2, 1))],
                51: [(emit_qT, (3, 1))],
                45: [(emit_final, (0, 0))],
                47: [(emit_final, (0, 1))],
                49: [(emit_final, (1, 0))],
                53: [(emit_final, (1, 1))],
                55: [(emit_final, (2, 0))],
                57: [(emit_final, (2, 1))],
                59: [(emit_final, (3, 0))],
                61: [(emit_final, (3, 1))],
            }

            # prologue inside the input-DMA shadow
            emit_qT(0, 0)
            emit_kT(0, 0)
            emit_kT(0, 1)
            emit_v(0)

            for s in range(NDS + LAG_D):
                if s < NDS:
                    emit_scores(s)
                a = s - LAG_D
                if a >= 0:
                    emit_av(a)
                    if a % 8 == 7:
                        emit_norm(a // 8)
                for fn, args in filler.get(s, []):
                    fn(*args)
                for step in FOLD.get(s, []):
                    run_fold(step)

            # remaining output tiles (query block 1; needs the last norms)
            for nt in range(4, NS // 128):
                for qb in range(QD // 512):
                    emit_final(nt, qb)
    nc.compile()
    return nc


def _get_nc():
    global _CACHED_NC
    if _CACHED_NC is None:
        _CACHED_NC = build_nc()
    return _CACHED_NC


def _prearrange(a2d, kchunks, cols):
    """[kchunks*128, cols] -> [128, kchunks*cols] with (p, k*cols+c) =
    a2d[k*128+p, c]."""
    return np.ascontiguousarray(
        a2d.reshape(kchunks, 128, cols).transpose(1, 0, 2).reshape(
            128, kchunks * cols))


def _shard_inputs(x, context, Wq, Wk, Wv, Wo, bo):
    import ml_dtypes
    bf = ml_dtypes.bfloat16
    Wq_p = _prearrange((np.asarray(Wq, np.float32) * SCALE).astype(bf),
                       8, INNER)
    Wk_p = _prearrange(np.asarray(Wk).astype(bf), 6, INNER)
    Wv_p = _prearrange(np.asarray(Wv).astype(bf), 6, INNER)
    Wo_p = _prearrange(np.asarray(Wo).astype(bf), 4, QD)
    bo_p = np.ascontiguousarray(
        np.broadcast_to(np.asarray(bo).astype(bf), (128, QD)))
    in_maps = []
    for c in range(8):
        b, q = divmod(c, 2)
        xT = np.asarray(x[b, q * NS:(q + 1) * NS, :]).astype(bf).T  # [QD, NS]
        ctxT = np.asarray(context[b]).astype(bf).T                  # [CD, M]
        im = {
            "xT0": _prearrange(np.ascontiguousarray(xT[:, 0:512]), 8, 512),
            "xT1": _prearrange(np.ascontiguousarray(xT[:, 512:1024]), 8, 512),
            "Wq": Wq_p, "Wk": Wk_p, "Wv": Wv_p, "Wo": Wo_p, "bo": bo_p,
        }
        for i in range(4):
            im[f"ctx{i}"] = _prearrange(
                np.ascontiguousarray(ctxT[:, i * 512:(i + 1) * 512]), 6, 512)
        in_maps.append(im)
    return in_maps


def kernel(x, context, Wq, Wk, Wv, Wo, bo, _trace=False):
    from concourse.bass_utils import run_bass_kernel_spmd

    nc = _get_nc()
    in_maps = _shard_inputs(x, context, Wq, Wk, Wv, Wo, bo)
    res = run_bass_kernel_spmd(nc, in_maps, core_ids=list(range(8)),
                               trace=_trace)
    out = np.empty((B, N, QD), np.float32)
    for c in range(8):
        b, q = divmod(c, 2)
        out[b, q * NS:(q + 1) * NS, :] = np.asarray(
            res.results[c]["out"], dtype=np.float32)
    if _trace:
        kernel._last_result = res
    return out


# revision 26
# speedup vs baseline: 1.0227x; 1.0227x over previous
"""Cross-attention kernel for Trainium2, SPMD across 8 NeuronCores.

Problem: B=4, N=M=2048, QD=1024, CD=768, H=8, DH=64, INNER=512 (f32).
  q = x @ Wq; k = ctx @ Wk; v = ctx @ Wv
  out = softmax(q k^T / sqrt(DH)) v @ Wo + bo

Sharding: batch x query-halves -> 8 shards. Core c handles batch c//2,
query rows (c%2)*1024:(c%2+1)*1024, with that batch's full context.
Weights replicated, cast to bf16 on host (scale folded into Wq).

Per-core dataflow (host pre-arranges every input into its exact SBUF
layout so each DMA is a flat contiguous copy; input DMAs split across
the SP and ACT hardware-DGE queues for parallel issue):
  qT = (scale*Wq)^T @ xT       [INNER, n]   bf16
  kT = Wk^T @ ctxT             [INNER, m]   bf16
  v  = ctxT^T @ Wv             [m, INNER]   bf16
  sT_h = kT_h^T q_h            [m, n] scores; head pairs share the PE
     via 64-row tiling.
  E = exp(sT) on ScalarE, [128,1024] per head per double-slot.
  Softmax denominators WITHOUT PE ones-matmuls: DVE folds the 8 E
     tiles of a group elementwise (bf16), GpSimd partition_all_reduce
     sums across partitions (f32), DVE reciprocal -> rb. This frees a
     full third of the attention PE streaming vs the ones-matmul way.
  O'_pair = v^T @ E per head pair via two concurrent 64-column tiles.
  O = O' * rb                  packed per INNER tile
  out = sum_j O_j^T @ Wo_j + bo (psum-drain DVE add, bf16 out DMA,
     host converts back to f32).

Schedule: 64 double-slots, groups ordered nb-major (all four head
pairs for query block 0, then block 1) so kT/v are produced once in
the front half and the back half is filled with the first block's
output projections. A.V lags scores by LAG_D=12 slots. Fillers are
placed so the PE slot period tracks ScalarE's exp pace (the binding
engine), and DVE fold work is spread one op per slot.
"""

import numpy as np

B, N, M = 4, 2048, 2048
QD, CD = 1024, 768
H, DH = 8, 64
INNER = H * DH  # 512
NS = 1024  # query rows per core
SCALE = DH ** -0.5

_CACHED_NC = None


def build_nc():
    import concourse.bacc as bacc
    import concourse.mybir as mybir
    import concourse.tile as tile
    from concourse import bass_isa

    f32 = mybir.dt.float32
    bf16 = mybir.dt.bfloat16
    FT = mybir.ActivationFunctionType
    AluOp = mybir.AluOpType
    ROp = bass_isa.ReduceOp

    nc = bacc.Bacc(None)
    # flat, host-prearranged inputs: [128 partitions, k-chunks * cols]
    xT0_d = nc.dram_tensor("xT0", (128, 8 * 512), bf16, kind="ExternalInput")
    xT1_d = nc.dram_tensor("xT1", (128, 8 * 512), bf16, kind="ExternalInput")
    ctx_d = [nc.dram_tensor(f"ctx{i}", (128, 6 * 512), bf16,
                            kind="ExternalInput") for i in range(4)]
    Wq_d = nc.dram_tensor("Wq", (128, 8 * 512), bf16, kind="ExternalInput")
    Wk_d = nc.dram_tensor("Wk", (128, 6 * 512), bf16, kind="ExternalInput")
    Wv_d = nc.dram_tensor("Wv", (128, 6 * 512), bf16, kind="ExternalInput")
    Wo_d = nc.dram_tensor("Wo", (128, 4 * 1024), bf16, kind="ExternalInput")
    bo_d = nc.dram_tensor("bo", (128, QD), bf16, kind="ExternalInput")
    out_d = nc.dram_tensor("out", (NS, QD), bf16, kind="ExternalOutput")

    KQ = QD // 128     # 8 k-tiles, q projection
    KC = CD // 128     # 6 k-tiles, k/v projections
    NI = INNER // 128  # 4 partition tiles of INNER (head pairs)
    MT = M // 128      # 16 context m-tiles
    NB = NS // 512     # 2 query blocks
    LAG_D = 12         # A.V lag in double-slots

    with tile.TileContext(nc) as tc, nc.allow_low_precision(
            "bf16 partial sums for softmax denominators; final reduce in f32"):
        with (
            tc.tile_pool(name="w", bufs=1) as wp,
            tc.tile_pool(name="a", bufs=1) as ap,
            tc.tile_pool(name="e", bufs=26) as ep,
            tc.tile_pool(name="ac", bufs=4) as accp,
            tc.tile_pool(name="h", bufs=2) as hp,
            tc.tile_pool(name="r", bufs=2) as rp,
            tc.tile_pool(name="rb", bufs=4) as rbp,
            tc.tile_pool(name="o", bufs=4) as op_,
            tc.tile_pool(name="pp", bufs=2, space="PSUM") as pp,
            tc.tile_pool(name="po", bufs=2, space="PSUM") as ppo,
            tc.tile_pool(name="pss", bufs=2, space="PSUM") as pps,
        ):
            # ---- input DMAs: flat copies; first-needed first.
            # sync (SP) queue carries the query-side stream, scalar (ACT)
            # queue carries the context-weight stream in parallel.
            wq_sb = wp.tile([128, 8 * 512], bf16, tag="wq", name="wq")
            xT_h = [wp.tile([128, 8 * 512], bf16, tag=f"xs{i}", name=f"xs{i}")
                    for i in range(2)]
            wk_sb = wp.tile([128, 6 * 512], bf16, tag="wk", name="wk")
            ctx_q = [wp.tile([128, 6 * 512], bf16, tag=f"cs{i}", name=f"cs{i}")
                     for i in range(4)]
            wv_sb = wp.tile([128, 6 * 512], bf16, tag="wv", name="wv")
            wo_sb = wp.tile([128, 4 * 1024], bf16, tag="wo", name="wo")
            bo_sb = wp.tile([128, QD], bf16, tag="bo", name="bo_sb")

            # input DMAs split across the two hardware-DGE queues (SP and
            # ACT), ordered by first use; the ACT queue finishes its issues
            # well before the first exp is ready to run.
            nc.sync.dma_start(wq_sb[:], Wq_d[:])
            nc.sync.dma_start(xT_h[0][:], xT0_d[:])
            nc.scalar.dma_start(wk_sb[:], Wk_d[:])
            nc.scalar.dma_start(ctx_q[0][:], ctx_d[0][:])
            nc.sync.dma_start(ctx_q[1][:], ctx_d[1][:])
            nc.scalar.dma_start(wv_sb[:], Wv_d[:])
            nc.sync.dma_start(ctx_q[2][:], ctx_d[2][:])
            nc.scalar.dma_start(ctx_q[3][:], ctx_d[3][:])
            nc.sync.dma_start(xT_h[1][:], xT1_d[:])
            nc.scalar.dma_start(wo_sb[:], Wo_d[:])
            nc.scalar.dma_start(bo_sb[:], bo_d[:])

            def wqs(k, j):
                return wq_sb[:, k * INNER + j * 128:k * INNER + (j + 1) * 128]

            def xts(k, nb):
                return xT_h[nb][:, k * 512:(k + 1) * 512]

            def wks(k, j):
                return wk_sb[:, k * INNER + j * 128:k * INNER + (j + 1) * 128]

            def ctxs(k, lo, sz):
                q, l2 = divmod(lo, M // 4)
                return ctx_q[q][:, k * (M // 4) + l2:k * (M // 4) + l2 + sz]

            def wvs(k):
                return wv_sb[:, k * INNER:(k + 1) * INNER]

            def wos(j, qb):
                return wo_sb[:, j * QD + qb * 512:j * QD + (qb + 1) * 512]

            # persistent activations
            qT = [[ap.tile([128, 512], bf16, tag=f"qT{j}_{nb}",
                           name=f"qT{j}_{nb}") for nb in range(NB)]
                  for j in range(NI)]
            kT = [[ap.tile([128, 512], bf16, tag=f"kT{j}_{mb}",
                           name=f"kT{j}_{mb}") for mb in range(4)]
                  for j in range(NI)]
            v = [ap.tile([128, INNER], bf16, tag=f"v{t}", name=f"v{t}")
                 for t in range(MT)]
            On = [ap.tile([128, NS], bf16, tag=f"On{j}", name=f"On{j}")
                  for j in range(NI)]

            def emit_qT(j, nb):
                ps = pp.tile([128, 512], f32, tag="pp", name="pp")
                for k in range(KQ):
                    nc.tensor.matmul(ps[:], wqs(k, j), xts(k, nb),
                                     start=(k == 0), stop=(k == KQ - 1))
                nc.vector.tensor_copy(qT[j][nb][:], ps[:])

            def emit_kT(j, mb):
                ps = pp.tile([128, 512], f32, tag="pp", name="pp")
                for k in range(KC):
                    nc.tensor.matmul(ps[:], wks(k, j), ctxs(k, mb * 512, 512),
                                     start=(k == 0), stop=(k == KC - 1))
                nc.vector.tensor_copy(kT[j][mb][:], ps[:])

            def emit_v(t):
                ps = pp.tile([128, 512], f32, tag="pp", name="pp")
                for k in range(KC):
                    nc.tensor.matmul(ps[:], ctxs(k, t * 128, 128), wvs(k),
                                     start=(k == 0), stop=(k == KC - 1))
                nc.vector.tensor_copy(v[t][:], ps[:])

            # groups nb-major: kT/v land in the front half, the back half
            # is topped up with query-block-0 output projections
            groups = [(j, nb) for nb in range(NB) for j in range(NI)]
            NDS = len(groups) * (MT // 2)  # 64 double-slots
            E = {}     # slot -> (E_h0, E_h1) [128, 1024] bf16
            PO = {}    # group idx -> [128, 512] psum (both heads packed)
            ACC = {}   # group idx -> [acc_h0, acc_h1] bf16 partial sums
            RB = {}    # group idx -> [128, 512] f32 reciprocal denominators

            def emit_scores(s):
                j, nb = groups[s // 8]
                p = s % 8
                psab = [pps.tile([128, 1024], f32, tag="pss", name="pss")
                        for _ in range(2)]
                for dt in range(2):
                    t = 2 * p + dt
                    for hh in range(2):
                        nc.tensor.matmul(
                            psab[hh][:, dt * 512:(dt + 1) * 512],
                            kT[j][t // 4][hh * 64:(hh + 1) * 64,
                                          (t % 4) * 128:(t % 4 + 1) * 128],
                            qT[j][nb][hh * 64:(hh + 1) * 64, :],
                            start=True, stop=True)
                es = []
                for hh in range(2):
                    e = ep.tile([128, 1024], bf16, tag="E", name="E")
                    nc.scalar.activation(e[:], psab[hh][:], FT.Exp)
                    es.append(e)
                E[s] = es

            def emit_av(a):
                gi = a // 8
                j, nb = groups[gi]
                p = a % 8
                if p == 0:
                    PO[gi] = ppo.tile([128, 512], f32, tag="po", name="po")
                po = PO[gi]
                for dt in range(2):
                    t = 2 * p + dt
                    st, sp_ = (t == 0), (t == MT - 1)
                    for hh in range(2):
                        h = 2 * j + hh
                        nc.tensor.matmul(
                            po[hh * 64:(hh + 1) * 64, :],
                            v[t][:, h * 64:(h + 1) * 64],
                            E[a][hh][:, dt * 512:(dt + 1) * 512],
                            start=st, stop=sp_, skip_group_check=True,
                            tile_position=(0, 64 * hh))
                del E[a]

            # denominator fold ops, spread one-ish per slot:
            #  pair:  acc = E[8g] + E[8g+1]          (slot 8g+2)
            #  add i: acc += E[8g+i], i=2..7         (slot 8g+i+1)
            #  half:  hsum = acc[:, :512]+acc[:, 512:]; r = allreduce(hsum)
            #  recip: rb[0:64] = 1/r_h0; rb[64:128] = 1/r_h1
            FOLD = {}
            for g in range(len(groups)):
                FOLD.setdefault(8 * g + 2, []).append(("pair", g))
                for i in range(2, 8):
                    FOLD.setdefault(8 * g + i + 1, []).append(("add", g, i))
                FOLD.setdefault(8 * g + 9, []).append(("half", g))
                FOLD.setdefault(8 * g + 10, []).append(("recip", g))

            def run_fold(step):
                kind, g = step[0], step[1]
                if kind == "pair":
                    acc = [accp.tile([128, 1024], bf16, tag="acc", name="acc")
                           for _ in range(2)]
                    for hh in range(2):
                        nc.vector.tensor_tensor(
                            acc[hh][:], E[8 * g][hh][:], E[8 * g + 1][hh][:],
                            op=AluOp.add)
                    ACC[g] = acc
                elif kind == "add":
                    i = step[2]
                    for hh in range(2):
                        nc.vector.tensor_tensor(
                            ACC[g][hh][:], ACC[g][hh][:], E[8 * g + i][hh][:],
                            op=AluOp.add)
                elif kind == "half":
                    hs = [hp.tile([128, 512], f32, tag="hs", name="hs")
                          for _ in range(2)]
                    rr = [rp.tile([128, 512], f32, tag="r", name="r")
                          for _ in range(2)]
                    for hh in range(2):
                        nc.vector.tensor_tensor(
                            hs[hh][:], ACC[g][hh][:, 0:512],
                            ACC[g][hh][:, 512:1024], op=AluOp.add)
                        nc.gpsimd.partition_all_reduce(
                            rr[hh][:], hs[hh][:], channels=128,
                            reduce_op=ROp.add)
                    ACC[g + 1000] = rr  # stash
                    del ACC[g]
                elif kind == "recip":
                    # full-tile recip: the custom DVE op breaks on
                    # partition-offset slices
                    rr = ACC.pop(g + 1000)
                    rbs = [rbp.tile([128, 512], f32, tag="rb", name="rb")
                           for _ in range(2)]
                    nc.vector.reciprocal_approx_fast(rbs[0][:], rr[0][:])
                    nc.vector.reciprocal_approx_fast(rbs[1][:], rr[1][:])
                    RB[g] = rbs

            def emit_norm(gi):
                j, nb = groups[gi]
                for hh in range(2):
                    sl = slice(hh * 64, (hh + 1) * 64)
                    nc.vector.tensor_tensor(
                        On[j][sl, nb * 512:(nb + 1) * 512], PO[gi][sl, :],
                        RB[gi][hh][sl, :], op=AluOp.mult)
                del PO[gi], RB[gi]

            def emit_final(nt, qb):
                pf = pp.tile([128, 512], f32, tag="pp", name="pf")
                for j in range(NI):
                    nc.tensor.matmul(
                        pf[:], On[j][:, nt * 128:(nt + 1) * 128],
                        wos(j, qb), start=(j == 0), stop=(j == NI - 1))
                ot = op_.tile([128, 512], bf16, tag="ot", name="ot")
                nc.vector.tensor_tensor(
                    ot[:], pf[:], bo_sb[:, qb * 512:(qb + 1) * 512],
                    op=AluOp.add)
                nc.sync.dma_start(
                    out_d[nt * 128:(nt + 1) * 128,
                          qb * 512:(qb + 1) * 512], ot[:])

            # fillers: slot -> emit thunks (deadlines derived from the
            # nb-major group order and LAG_D; see transcript analysis)
            filler = {
                0: [(emit_v, (1,)), (emit_kT, (0, 2))],
                1: [(emit_v, (2,)), (emit_kT, (0, 3))],
                2: [(emit_v, (3,)), (emit_kT, (1, 0))],
                3: [(emit_v, (4,)), (emit_qT, (1, 0))],
                4: [(emit_v, (5,)), (emit_kT, (1, 1))],
                5: [(emit_v, (6,)), (emit_kT, (1, 2))],
                6: [(emit_v, (7,)), (emit_kT, (1, 3))],
                7: [(emit_v, (8,))],
                8: [(emit_v, (9,))],
                9: [(emit_v, (10,)), (emit_kT, (2, 0))],
                10: [(emit_v, (11,)), (emit_qT, (2, 0))],
                11: [(emit_v, (12,)), (emit_kT, (2, 1))],
                12: [(emit_v, (13,)), (emit_kT, (2, 2))],
                13: [(emit_v, (14,)), (emit_kT, (2, 3))],
                14: [(emit_v, (15,))],
                15: [(emit_kT, (3, 0))],
                16: [(emit_kT, (3, 1))],
                17: [(emit_kT, (3, 2))],
                18: [(emit_kT, (3, 3))],
                19: [(emit_qT, (3, 0))],
                27: [(emit_qT, (0, 1))],
                35: [(emit_qT, (1, 1))],
                43: [(emit_qT, (2, 1))],
                51: [(emit_qT, (3, 1))],
                45: [(emit_final, (0, 0))],
                47: [(emit_final, (0, 1))],
                49: [(emit_final, (1, 0))],
                53: [(emit_final, (1, 1))],
                55: [(emit_final, (2, 0))],
                57: [(emit_final, (2, 1))],
                59: [(emit_final, (3, 0))],
                61: [(emit_final, (3, 1))],
            }

            # prologue inside the input-DMA shadow
            emit_qT(0, 0)
            emit_kT(0, 0)
            emit_kT(0, 1)
            emit_v(0)

            for s in range(NDS + LAG_D):
                if s < NDS:
                    emit_scores(s)
                a = s - LAG_D
                if a >= 0:
                    emit_av(a)
                    if a % 8 == 7:
                        emit_norm(a // 8)
                for fn, args in filler.get(s, []):
                    fn(*args)
                for step in FOLD.get(s, []):
                    run_fold(step)

            # remaining output tiles (query block 1; needs the last norms)
            for nt in range(4, NS // 128):
                for qb in range(QD // 512):
                    emit_final(nt, qb)
    nc.compile()
    return nc


def _get_nc():
    global _CACHED_NC
    if _CACHED_NC is None:
        _CACHED_NC = build_nc()
    return _CACHED_NC


def _prearrange(a2d, kchunks, cols):
    """[kchunks*128, cols] -> [128, kchunks*cols] with (p, k*cols+c) =
    a2d[k*128+p, c]."""
    return np.ascontiguousarray(
        a2d.reshape(kchunks, 128, cols).transpose(1, 0, 2).reshape(
            128, kchunks * cols))


def _shard_inputs(x, context, Wq, Wk, Wv, Wo, bo):
    import ml_dtypes
    bf = ml_dtypes.bfloat16
    Wq_p = _prearrange((np.asarray(Wq, np.float32) * SCALE).astype(bf),
                       8, INNER)
    Wk_p = _prearrange(np.asarray(Wk).astype(bf), 6, INNER)
    Wv_p = _prearrange(np.asarray(Wv).astype(bf), 6, INNER)
    Wo_p = _prearrange(np.asarray(Wo).astype(bf), 4, QD)
    bo_p = np.ascontiguousarray(
        np.broadcast_to(np.asarray(bo).astype(bf), (128, QD)))
    in_maps = []
    for c in range(8):
        b, q = divmod(c, 2)
        xT = np.asarray(x[b, q * NS:(q + 1) * NS, :]).astype(bf).T  # [QD, NS]
        ctxT = np.asarray(context[b]).astype(bf).T                  # [CD, M]
        im = {
            "xT0": _prearrange(np.ascontiguousarray(xT[:, 0:512]), 8, 512),
            "xT1": _prearrange(np.ascontiguousarray(xT[:, 512:1024]), 8, 512),
            "Wq": Wq_p, "Wk": Wk_p, "Wv": Wv_p, "Wo": Wo_p, "bo": bo_p,
        }
        for i in range(4):
            im[f"ctx{i}"] = _prearrange(
                np.ascontiguousarray(ctxT[:, i * 512:(i + 1) * 512]), 6, 512)
        in_maps.append(im)
    return in_maps


def kernel(x, context, Wq, Wk, Wv, Wo, bo, _trace=False):
    from concourse.bass_utils import run_bass_kernel_spmd

    nc = _get_nc()
    in_maps = _shard_inputs(x, context, Wq, Wk, Wv, Wo, bo)
    res = run_bass_kernel_spmd(nc, in_maps, core_ids=list(range(8)),
                               trace=_trace)
    out = np.empty((B, N, QD), np.float32)
    for c in range(8):
        b, q = divmod(c, 2)
        out[b, q * NS:(q + 1) * NS, :] = np.asarray(
            res.results[c]["out"], dtype=np.float32)
    if _trace:
        kernel._last_result = res
    return out
